# revision 4
# baseline (speedup 1.0000x reference)
"""CentroidLoss Trainium2 kernel.

Data-parallel over the batch on 8 NeuronCores; centers replicated.

Reference math (B=8192, D=128, K=256, RHO=1.0):
    norm_sq[b,k] = ||h_b - c_k||^2
    distance[b]  = norm_sq[b, y_b]
    logsum[b]    = logsumexp_k(-sqrt(norm_sq[b,:]))
    loss = mean(distance + logsum)
    pd[i,j] = ||c_i - c_j||^2 (diag -> inf)
    reg  = sum_i(-min_j(log(pd[i,j])))        == -sum_i log(min_j pd[i,j])
    out  = loss + RHO * reg

Per-core kernel layout (batch shard Bs=1024, 8 tiles of 128 rows):
    psum  = hT_i^T @ (-2 cT)  (+ ones x ccrow 1-row matmul)   -> cc[k]-2 h.c
    t     = Sqrt(psum + hh_col)        (ACT, per-partition bias)
    e     = Exp(-t), accum_out -> S8[:, i]  (logsumexp denominator)
    oh    = (iota == y_col_i)          (GPSIMD one-hot)
    g     = sum_k t*oh  (DVE tensor_tensor_reduce) -> G8[:, i]   (= sqrt(dist))
  Epilogue:  Square(G8) accum -> dist sums; Ln(S8) accum -> lse sums;
             reg via same matmul trick + affine_select diag mask + min + Ln;
             final ones-matmul partition reduce -> out[2,1] per core.
Host: loss = sum(out[c][0])/B - RHO * out[0][1].
"""

import numpy as np

B, D, K = 8192, 128, 256
RHO = 1.0
N_CORES = 8
BS = B // N_CORES      # 1024 rows per core
P = 128                # partitions / tile rows
NT = BS // P           # 8 batch tiles per core
KBLK = K // P          # 2 center blocks for the reg term
DIAG_BIG = 1.0e30      # masks the pd diagonal out of the min


def _build():
    import concourse.bacc as bacc
    import concourse.tile as tile
    import concourse.mybir as mybir
    from concourse import bass

    f32 = mybir.dt.float32
    nc = bacc.Bacc(None, target_bir_lowering=False)

    # DRAM I/O (per-core shard shapes; host does the shard/replicate split).
    # ht3[i] is the i-th batch tile of h^T, stored tile-contiguous.
    ht_d = nc.dram_tensor("ht", [NT, D, P], f32, kind="ExternalInput")
    yf_d = nc.dram_tensor("yf", [P, NT], f32, kind="ExternalInput")
    ct_d = nc.dram_tensor("ct", [D, K], f32, kind="ExternalInput")
    ccr_d = nc.dram_tensor("ccr", [1, K], f32, kind="ExternalInput")
    cc2_d = nc.dram_tensor("cc2", [P, KBLK], f32, kind="ExternalInput")
    out_d = nc.dram_tensor("out", [2, 1], f32, kind="ExternalOutput")

    AF = mybir.ActivationFunctionType
    ALU = mybir.AluOpType
    AX = mybir.AxisListType

    with tile.TileContext(nc) as tc:
        with (
            tc.tile_pool(name="const", bufs=1) as const,
            tc.tile_pool(name="work", bufs=3) as work,
            tc.tile_pool(name="psum", bufs=3, space="PSUM") as psum,
            tc.tile_pool(name="psmall", bufs=2, space="PSUM") as psmall,
        ):
            # ---- constants / replicated inputs ----
            ones_col = const.tile([P, 1], f32)
            nc.vector.memset(ones_col, 1.0)
            ones_row = const.tile([1, P], f32)
            nc.vector.memset(ones_row, 1.0)

            iota_f = const.tile([P, K], f32)
            nc.gpsimd.iota(iota_f, pattern=[[1, K]], base=0,
                           channel_multiplier=0,
                           allow_small_or_imprecise_dtypes=True)

            ct_sb = const.tile([D, K], f32)
            nc.sync.dma_start(out=ct_sb, in_=ct_d[:, :])
            ctm2 = const.tile([D, K], f32)
            nc.scalar.mul(ctm2, ct_sb, -2.0)

            ccr_sb = const.tile([1, K], f32)
            nc.sync.dma_start(out=ccr_sb, in_=ccr_d[:, :])
            cc2_sb = const.tile([P, KBLK], f32)
            nc.sync.dma_start(out=cc2_sb, in_=cc2_d[:, :])
            yf_sb = const.tile([P, NT], f32)
            nc.sync.dma_start(out=yf_sb, in_=yf_d[:, :])

            G8 = const.tile([P, NT], f32)   # sqrt(distance) per tile col
            S8 = const.tile([P, NT], f32)   # sum_k exp(-t) per tile col
            final2 = const.tile([P, 2], f32)  # [:,0] dist+lse, [:,1] reg

            # ---- main batch loop ----
            for i in range(NT):
                ht_i = work.tile([D, P], f32, tag="ht")
                nc.sync.dma_start(out=ht_i, in_=ht_d[i, :, :])

                # hh[b] = sum_d h^2 : square then PE column-sum
                sq_i = work.tile([D, P], f32, tag="sq")
                nc.gpsimd.tensor_mul(sq_i, ht_i, ht_i)
                ps_hh = psmall.tile([P, 1], f32, tag="ps_hh")
                nc.tensor.matmul(ps_hh, sq_i, ones_col, start=True, stop=True)
                hh_i = work.tile([P, 1], f32, tag="hh")
                nc.scalar.copy(hh_i, ps_hh)

                # cc[k] - 2 h.c  accumulated in PSUM
                ps_i = psum.tile([P, K], f32, tag="mm")
                nc.tensor.matmul(ps_i, ht_i, ctm2, start=True, stop=False)
                nc.tensor.matmul(ps_i, ones_row, ccr_sb, start=False, stop=True)

                # t = sqrt(norm_sq), e = exp(-t) with fused row-sum
                t_i = work.tile([P, K], f32, tag="t")
                nc.scalar.activation(t_i, ps_i, AF.Sqrt, bias=hh_i, scale=1.0)
                e_i = work.tile([P, K], f32, tag="e")
                nc.scalar.activation(e_i, t_i, AF.Exp, scale=-1.0,
                                     accum_out=S8[:, i:i + 1])

                # fused one-hot gather: (iota == y) * t, summed over k
                tj_i = work.tile([P, K], f32, tag="tj")
                nc.vector.scalar_tensor_tensor(
                    out=tj_i, in0=iota_f, scalar=yf_sb[:, i:i + 1], in1=t_i,
                    op0=ALU.is_equal, op1=ALU.mult,
                    accum_out=G8[:, i:i + 1])

            # ---- regularization (redundant on every core) ----
            regmin = const.tile([P, KBLK], f32)
            for b in range(KBLK):
                ps_r = psum.tile([P, K], f32, tag="mm")
                nc.tensor.matmul(ps_r, ct_sb[:, b * P:(b + 1) * P], ctm2,
                                 start=True, stop=False)
                nc.tensor.matmul(ps_r, ones_row, ccr_sb, start=False, stop=True)
                pd_b = work.tile([P, K], f32, tag="pd")
                nc.vector.tensor_scalar_add(pd_b, ps_r, cc2_sb[:, b:b + 1])
                pdm_b = work.tile([P, K], f32, tag="pdm")
                # iota = b*128 + p - f == 0 exactly on the diagonal
                nc.gpsimd.affine_select(
                    out=pdm_b, in_=pd_b, compare_op=ALU.not_equal,
                    fill=DIAG_BIG, base=b * P, channel_multiplier=1,
                    pattern=[[-1, K]])
                nc.vector.tensor_reduce(regmin[:, b:b + 1], pdm_b,
                                        axis=AX.X, op=ALU.min)

            # ---- epilogue: fold everything to two scalars ----
            sq8 = const.tile([P, NT], f32)
            dcol = const.tile([P, 1], f32)
            nc.scalar.activation(sq8, G8, AF.Square, accum_out=dcol)
            ln8 = const.tile([P, NT], f32)
            lcol = const.tile([P, 1], f32)
            nc.scalar.activation(ln8, S8, AF.Ln, accum_out=lcol)
            nc.vector.tensor_add(final2[:, 0:1], dcol, lcol)

            lnr = const.tile([P, KBLK], f32)
            nc.scalar.activation(lnr, regmin, AF.Ln,
                                 accum_out=final2[:, 1:2])

            ps_f = psmall.tile([2, 1], f32, tag="ps_f")
            nc.tensor.matmul(ps_f, final2, ones_col, start=True, stop=True)
            out_sb = const.tile([2, 1], f32)
            nc.scalar.copy(out_sb, ps_f)
            nc.sync.dma_start(out=out_d[:, :], in_=out_sb)

    return nc


def _shard_inputs(h, y, centers):
    """Full inputs -> per-core in_maps (host-side shard/replicate only)."""
    h = np.asarray(h, dtype=np.float32)
    y = np.asarray(y)
    centers = np.asarray(centers, dtype=np.float32)

    ct = np.ascontiguousarray(centers.T)                      # (D, K)
    cc = np.sum(centers.astype(np.float32) ** 2, axis=1)      # (K,)
    ccr = np.ascontiguousarray(cc[None, :])                   # (1, K)
    cc2 = np.ascontiguousarray(cc.reshape(KBLK, P).T)         # (P, KBLK)

    in_maps = []
    for c in range(N_CORES):
        hs = h[c * BS:(c + 1) * BS]                           # (BS, D)
        ys = y[c * BS:(c + 1) * BS].astype(np.float32)        # exact: 0..K-1
        ht3 = np.ascontiguousarray(
            hs.T.reshape(D, NT, P).transpose(1, 0, 2))        # (NT, D, P)
        yf = np.ascontiguousarray(ys.reshape(NT, P).T)        # (P, NT)
        in_maps.append({
            "ht": ht3, "yf": yf, "ct": ct, "ccr": ccr, "cc2": cc2,
        })
    return in_maps


_NC_CACHE = {}


def kernel(h, y, centers):
    from concourse.bass_utils import run_bass_kernel_spmd

    if "nc" not in _NC_CACHE:
        nc = _build()
        nc.finalize()
        _NC_CACHE["nc"] = nc
    nc = _NC_CACHE["nc"]

    in_maps = _shard_inputs(h, y, centers)
    res = run_bass_kernel_spmd(nc, in_maps, core_ids=list(range(N_CORES)))

    part = np.stack([r["out"].reshape(2) for r in res.results])  # (8, 2)
    loss_mean = float(np.sum(part[:, 0], dtype=np.float64)) / B
    reg = -float(part[0, 1])
    return np.float32(loss_mean + RHO * reg)


# revision 11
# speedup vs baseline: 1.1100x; 1.1100x over previous
"""CentroidLoss Trainium2 kernel (v2).

Data-parallel over the batch on 8 NeuronCores; centers replicated.

Per-core math (shard Bs=1024, 8 b-tiles of 128 rows, K=256, D=128):
    psum[b,k] = -2 h.c (bf16 mm) + cc_hi[k]+cc_lo[k] (2-row bf16 fold)
                + hh[b] (1-row bf16 fold)          == ||h_b - c_k||^2
    u = Ln(psum);  t = Exp(0.5u) == sqrt;  e = Exp(-t)   (one ACT table set,
    preloaded by a dummy Exp during the DMA prologue)
    distance[b] = psum[b, y_b] via DVE (iota==y)*psum fused gather
    lse[b] = Ln(sum_k e)  (DVE 3D reduce of e)
    reg: pd = cc_i + cc_j - 2 C C^T (same mm tricks), diag masked by
    affine_select, row min, Ln, sum.
Output per core: [sum_b(distance+lse), sum_i ln(min_j pd)] as (2,1) f32.
Host: loss = sum(out[:,0])/B - RHO * out[0,1].
"""

import numpy as np
import ml_dtypes

B, D, K = 8192, 128, 256
RHO = 1.0
N_CORES = 8
BS = B // N_CORES      # 1024 rows per core
P = 128
NT = BS // P           # 8 batch tiles per core
KBLK = K // P          # 2 center blocks for the reg term
DIAG_BIG = 1.0e30


def _build():
    import concourse.bacc as bacc
    import concourse.tile as tile
    import concourse.mybir as mybir
    from concourse import bass

    f32 = mybir.dt.float32
    f32r = mybir.dt.float32r
    bf16 = mybir.dt.bfloat16
    nc = bacc.Bacc(None, target_bir_lowering=False)

    ht_d = nc.dram_tensor("ht", [D, BS], bf16, kind="ExternalInput")
    yf_d = nc.dram_tensor("yf", [P, NT], f32, kind="ExternalInput")
    cth_d = nc.dram_tensor("cth", [D, K], bf16, kind="ExternalInput")
    ccr_d = nc.dram_tensor("ccr", [2, K], bf16, kind="ExternalInput")
    cc2_d = nc.dram_tensor("cc2", [P, KBLK], f32, kind="ExternalInput")
    out_d = nc.dram_tensor("out", [2, 1], f32, kind="ExternalOutput")

    AF = mybir.ActivationFunctionType
    ALU = mybir.AluOpType
    AX = mybir.AxisListType
    ts = bass.ts

    with tile.TileContext(nc) as tc:
        with (
            tc.tile_pool(name="const", bufs=1) as const,
            tc.tile_pool(name="work", bufs=2) as work,
            tc.tile_pool(name="pmm", bufs=1, space="PSUM") as pmm,
            tc.tile_pool(name="prg", bufs=2, space="PSUM") as prg,
            tc.tile_pool(name="psm", bufs=2, space="PSUM") as psm,
        ):
            # ---- constants / replicated inputs (ACT-ring DMAs) ----
            cth = const.tile([D, K], bf16)
            nc.scalar.dma_start(out=cth, in_=cth_d[:, :])
            ccr = const.tile([2, K], bf16)
            nc.scalar.dma_start(out=ccr, in_=ccr_d[:, :])
            cc2 = const.tile([P, KBLK], f32)
            nc.scalar.dma_start(out=cc2, in_=cc2_d[:, :])
            yf = const.tile([P, NT], f32)
            nc.scalar.dma_start(out=yf, in_=yf_d[:, :])

            # bulk h^T on the sync ring, two halves
            ht = const.tile([D, BS], bf16)
            nc.sync.dma_start(out=ht[:, 0:BS // 2], in_=ht_d[:, 0:BS // 2])
            nc.sync.dma_start(out=ht[:, BS // 2:], in_=ht_d[:, BS // 2:])

            # ACT table prefetch: dummy Exp loads natural_log_exp set early
            warm = const.tile([1, 1], f32)
            nc.vector.memset(warm, 1.0)
            nc.scalar.activation(warm, warm, AF.Exp)

            iota_f = const.tile([P, K], f32)
            nc.gpsimd.iota(iota_f, pattern=[[1, K]], base=0,
                           channel_multiplier=0,
                           allow_small_or_imprecise_dtypes=True)
            ones2 = const.tile([2, P], bf16)
            nc.vector.memset(ones2, 1.0)
            onec_r0 = const.tile([D, 1], f32)
            nc.vector.memset(onec_r0, 1.0)
            onec_r = onec_r0.bitcast(f32r)
            onec_f = const.tile([P, 1], f32)
            nc.vector.memset(onec_f, 1.0)
            oner_r0 = const.tile([1, K], f32)
            nc.vector.memset(oner_r0, 1.0)
            oner_r = oner_r0.bitcast(f32r)

            cm2 = const.tile([D, K], bf16)
            nc.vector.tensor_scalar_mul(cm2, cth, -2.0)

            sqh = const.tile([D, BS], f32r)
            nc.gpsimd.tensor_mul(sqh[:, 0:BS // 2], ht[:, 0:BS // 2],
                                 ht[:, 0:BS // 2])
            nc.gpsimd.tensor_mul(sqh[:, BS // 2:], ht[:, BS // 2:],
                                 ht[:, BS // 2:])

            # ---- reg matmuls early: fills PE while h loads ----
            regmin = const.tile([P, KBLK], f32)
            for bk in range(KBLK):
                psr = prg.tile([P, K], f32, tag="rg")
                nc.tensor.matmul(psr, cth[:, ts(bk, P)], cm2,
                                 start=True, stop=False)
                nc.tensor.matmul(psr, ones2, ccr,
                                 start=False, stop=True)
                pd = work.tile([P, K], f32, tag="pd")
                nc.vector.tensor_scalar_add(pd, psr, cc2[:, bk:bk + 1])
                pdm = work.tile([P, K], f32, tag="pdm")
                nc.gpsimd.affine_select(
                    out=pdm, in_=pd, compare_op=ALU.not_equal,
                    fill=DIAG_BIG, base=bk * P, channel_multiplier=1,
                    pattern=[[-1, K]])
                nc.vector.tensor_reduce(regmin[:, bk:bk + 1], pdm,
                                        axis=AX.X, op=ALU.min)

            # ---- hh rows: exact f32 column sums of sqh, two tiles/op ----
            hh_sb = const.tile([1, BS], f32r)
            for j in range(NT // 2):
                php = psm.tile([1, 2 * P], f32, tag="small")
                nc.tensor.matmul(php, onec_r, sqh[:, ts(j, 2 * P)],
                                 start=True, stop=True)
                nc.vector.tensor_copy(hh_sb[0:1, ts(j, 2 * P)], php)

            # ---- main: 8 b-tiles -> two (128,1024) psum groups ----
            NSLN = const.tile([P, 2 * NT], f32)   # gathers | ln(S8)
            S8 = const.tile([P, NT], f32)
            halves = [pmm.tile([P, 4 * K], f32, name=f"mm{j}", tag=f"mm{j}")
                      for j in range(2)]
            u_t = [const.tile([P, 4 * K], f32, name=f"u{j}", tag=f"u{j}")
                   for j in range(2)]
            t_t = [const.tile([P, 4 * K], f32, name=f"t{j}", tag=f"t{j}")
                   for j in range(2)]
            e_t = [const.tile([P, 4 * K], bf16, name=f"e{j}", tag=f"e{j}")
                   for j in range(2)]

            for h_i in range(2):
                pg = halves[h_i]
                for q in range(4):
                    i = 4 * h_i + q
                    quarter = pg[:, ts(q, K)]
                    nc.tensor.matmul(quarter, ht[:, ts(i, P)], cm2,
                                     start=True, stop=False)
                    nc.tensor.matmul(quarter, ones2, ccr,
                                     start=False, stop=False)
                    nc.tensor.matmul(quarter, hh_sb[0:1, ts(i, P)], oner_r,
                                     start=False, stop=True)
                    # fused gather: distance[b] = psum[b, y_b]
                    tj = work.tile([P, K], f32, tag="tj")
                    nc.vector.scalar_tensor_tensor(
                        out=tj, in0=iota_f, scalar=yf[:, i:i + 1],
                        in1=quarter, op0=ALU.is_equal, op1=ALU.mult,
                        accum_out=NSLN[:, i:i + 1])
                # ln -> exp(0.5) -> exp(-1) chain over the whole half
                nc.scalar.activation(u_t[h_i], pg, AF.Ln)
                nc.scalar.activation(t_t[h_i], u_t[h_i], AF.Exp, scale=0.5)
                nc.scalar.activation(e_t[h_i], t_t[h_i], AF.Exp, scale=-1.0)
                e3 = e_t[h_i].rearrange("p (n k) -> p n k", k=K)
                nc.vector.tensor_reduce(S8[:, ts(h_i, 4)], e3,
                                        axis=AX.X, op=ALU.add)

            # ---- epilogue ----
            nc.scalar.activation(NSLN[:, NT:2 * NT], S8, AF.Ln)
            rm = const.tile([P, KBLK], f32)
            nc.scalar.activation(rm, regmin, AF.Ln)

            final2 = const.tile([P, 2], f32)
            nc.vector.tensor_reduce(final2[:, 0:1], NSLN, axis=AX.X,
                                    op=ALU.add)
            nc.vector.tensor_reduce(final2[:, 1:2], rm, axis=AX.X,
                                    op=ALU.add)
            psf = psm.tile([2, 1], f32, tag="small")
            nc.tensor.matmul(psf, final2, onec_f, start=True, stop=True)
            out_sb = const.tile([2, 1], f32)
            nc.vector.tensor_copy(out_sb, psf)
            nc.sync.dma_start(out=out_d[:, :], in_=out_sb)

    return nc


def _shard_inputs(h, y, centers):
    """Full inputs -> per-core in_maps (host-side shard/replicate/cast)."""
    h = np.asarray(h, dtype=np.float32)
    y = np.asarray(y)
    centers = np.asarray(centers, dtype=np.float32)
    bf = ml_dtypes.bfloat16

    cth = np.ascontiguousarray(centers.T).astype(bf)           # (D, K)
    cc = np.sum(centers.astype(np.float64) ** 2, axis=1).astype(np.float32)
    cc_hi = cc.astype(bf)
    cc_lo = (cc - cc_hi.astype(np.float32)).astype(bf)
    ccr = np.stack([cc_hi, cc_lo]).astype(bf)                  # (2, K)
    cc2 = np.ascontiguousarray(cc.reshape(KBLK, P).T)          # (P, KBLK)

    in_maps = []
    for c in range(N_CORES):
        hs = h[c * BS:(c + 1) * BS]                            # (BS, D)
        ys = y[c * BS:(c + 1) * BS].astype(np.float32)
        ht = np.ascontiguousarray(hs.T).astype(bf)             # (D, BS)
        yf = np.ascontiguousarray(ys.reshape(NT, P).T)         # (P, NT)
        in_maps.append({
            "ht": ht, "yf": yf, "cth": cth, "ccr": ccr, "cc2": cc2,
        })
    return in_maps


_NC_CACHE = {}


def kernel(h, y, centers):
    from concourse.bass_utils import run_bass_kernel_spmd

    if "nc" not in _NC_CACHE:
        nc = _build()
        nc.finalize()
        _NC_CACHE["nc"] = nc
    nc = _NC_CACHE["nc"]

    in_maps = _shard_inputs(h, y, centers)
    res = run_bass_kernel_spmd(nc, in_maps, core_ids=list(range(N_CORES)))

    part = np.stack([r["out"].reshape(2) for r in res.results])  # (8, 2)
    loss_mean = float(np.sum(part[:, 0], dtype=np.float64)) / B
    reg = -float(part[0, 1])
    return np.float32(loss_mean + RHO * reg)


# revision 13
# speedup vs baseline: 1.2817x; 1.1547x over previous
"""CentroidLoss Trainium2 kernel (v2).

Data-parallel over the batch on 8 NeuronCores; centers replicated.

Per-core math (shard Bs=1024, 8 b-tiles of 128 rows, K=256, D=128):
    psum[b,k] = -2 h.c (bf16 mm) + cc_hi[k]+cc_lo[k] (2-row bf16 fold)
                + hh[b] (1-row bf16 fold)          == ||h_b - c_k||^2
    u = Ln(psum);  t = Exp(0.5u) == sqrt;  e = Exp(-t)   (one ACT table set,
    preloaded by a dummy Exp during the DMA prologue)
    distance[b] = psum[b, y_b] via DVE (iota==y)*psum fused gather
    lse[b] = Ln(sum_k e)  (DVE 3D reduce of e)
    reg: pd = cc_i + cc_j - 2 C C^T (same mm tricks), diag masked by
    affine_select, row min, Ln, sum.
Output per core: [sum_b(distance+lse), sum_i ln(min_j pd)] as (2,1) f32.
Host: loss = sum(out[:,0])/B - RHO * out[0,1].
"""

import numpy as np
import ml_dtypes

B, D, K = 8192, 128, 256
RHO = 1.0
N_CORES = 8
BS = B // N_CORES      # 1024 rows per core
P = 128
NT = BS // P           # 8 batch tiles per core
KBLK = K // P          # 2 center blocks for the reg term
DIAG_BIG = 1.0e30


def _build():
    import concourse.bacc as bacc
    import concourse.tile as tile
    import concourse.mybir as mybir
    from concourse import bass

    f32 = mybir.dt.float32
    f32r = mybir.dt.float32r
    bf16 = mybir.dt.bfloat16
    nc = bacc.Bacc(None, target_bir_lowering=False)

    ht_d = nc.dram_tensor("ht", [D, BS], bf16, kind="ExternalInput")
    # packed consts: cth (bf16 as 128 f32 cols) | cc2 (2) | yf (8)
    cpk_d = nc.dram_tensor("cpk", [P, K // 2 + KBLK + NT], f32,
                           kind="ExternalInput")
    ccr_d = nc.dram_tensor("ccr", [2, K], bf16, kind="ExternalInput")
    out_d = nc.dram_tensor("out", [2, 1], f32, kind="ExternalOutput")

    AF = mybir.ActivationFunctionType
    ALU = mybir.AluOpType
    AX = mybir.AxisListType
    ts = bass.ts

    with tile.TileContext(nc) as tc:
        with (
            tc.tile_pool(name="const", bufs=1) as const,
            tc.tile_pool(name="work", bufs=2) as work,
            tc.tile_pool(name="pmm", bufs=1, space="PSUM") as pmm,
            tc.tile_pool(name="prg", bufs=2, space="PSUM") as prg,
            tc.tile_pool(name="psm", bufs=2, space="PSUM") as psm,
        ):
            # ACT table preload: combined ln+exp set, hidden in prologue
            from concourse.hw_specs import get_activation_tables
            sid = list(get_activation_tables(nc.m.arch)).index(
                "natural_log_exp_and_others")
            if True:  # ATL disabled: dummy Ln warms the combined set
                warm = const.tile([1, 1], f32)
                nc.vector.memset(warm, 1.0)
                nc.scalar.activation(warm, warm, AF.Ln)

            # ---- constants / replicated inputs (ACT-ring DMAs) ----
            cpk = const.tile([P, K // 2 + KBLK + NT], f32)
            nc.scalar.dma_start(out=cpk, in_=cpk_d[:, :])
            cth = cpk[:, 0:K // 2].bitcast(bf16)
            cc2 = cpk[:, K // 2:K // 2 + KBLK]
            yf = cpk[:, K // 2 + KBLK:]
            ccr = const.tile([2, K], bf16)
            nc.scalar.dma_start(out=ccr, in_=ccr_d[:, :])

            # bulk h^T on the sync ring, two halves
            ht = const.tile([D, BS], bf16)
            nc.sync.dma_start(out=ht[:, 0:BS // 2], in_=ht_d[:, 0:BS // 2])
            nc.sync.dma_start(out=ht[:, BS // 2:], in_=ht_d[:, BS // 2:])

            iota_f = const.tile([P, K], f32)
            nc.gpsimd.iota(iota_f, pattern=[[1, K]], base=0,
                           channel_multiplier=0,
                           allow_small_or_imprecise_dtypes=True)
            ones2 = const.tile([2, P], bf16)
            nc.vector.memset(ones2, 1.0)
            onec_r0 = const.tile([D, 1], f32)
            nc.vector.memset(onec_r0, 1.0)
            onec_r = onec_r0.bitcast(f32r)
            onec_f = const.tile([P, 1], f32)
            nc.vector.memset(onec_f, 1.0)
            oner_r0 = const.tile([1, K], f32)
            nc.vector.memset(oner_r0, 1.0)
            oner_r = oner_r0.bitcast(f32r)

            cm2 = const.tile([D, K], bf16)
            nc.vector.tensor_scalar_mul(cm2, cth, -2.0)

            sqh = const.tile([D, BS], f32r)
            nc.gpsimd.tensor_mul(sqh[:, 0:BS // 2], ht[:, 0:BS // 2],
                                 ht[:, 0:BS // 2])
            nc.gpsimd.tensor_mul(sqh[:, BS // 2:], ht[:, BS // 2:],
                                 ht[:, BS // 2:])

            # ---- reg matmuls early: fills PE while h loads ----
            regmin = const.tile([P, KBLK], f32)
            for bk in range(KBLK):
                psr = prg.tile([P, K], f32, tag="rg")
                nc.tensor.matmul(psr, cth[:, ts(bk, P)], cm2,
                                 start=True, stop=False)
                nc.tensor.matmul(psr, ones2, ccr,
                                 start=False, stop=True)
                pd = work.tile([P, K], f32, tag="pd")
                nc.vector.tensor_scalar_add(pd, psr, cc2[:, bk:bk + 1])
                pdm = work.tile([P, K], f32, tag="pdm")
                nc.gpsimd.affine_select(
                    out=pdm, in_=pd, compare_op=ALU.not_equal,
                    fill=DIAG_BIG, base=bk * P, channel_multiplier=1,
                    pattern=[[-1, K]])
                nc.vector.tensor_reduce(regmin[:, bk:bk + 1], pdm,
                                        axis=AX.X, op=ALU.min)

            # ---- hh rows: exact f32 column sums of sqh, two tiles/op ----
            hh_sb = const.tile([1, BS], f32r)
            for j in range(NT // 2):
                php = psm.tile([1, 2 * P], f32, tag="small")
                nc.tensor.matmul(php, onec_r, sqh[:, ts(j, 2 * P)],
                                 start=True, stop=True)
                nc.vector.tensor_copy(hh_sb[0:1, ts(j, 2 * P)], php)

            # ---- main: 8 b-tiles -> two (128,1024) psum groups ----
            NSLN = const.tile([P, 2 * NT], f32)   # gathers | ln(S8)
            S8 = const.tile([P, NT], f32)
            halves = [pmm.tile([P, 4 * K], f32, name=f"mm{j}", tag=f"mm{j}")
                      for j in range(2)]
            u_t = [const.tile([P, 4 * K], f32, name=f"u{j}", tag=f"u{j}")
                   for j in range(2)]
            t_t = [const.tile([P, 4 * K], f32, name=f"t{j}", tag=f"t{j}")
                   for j in range(2)]
            e_t = [const.tile([P, 4 * K], bf16, name=f"e{j}", tag=f"e{j}")
                   for j in range(2)]

            for h_i in range(2):
                pg = halves[h_i]
                for q in range(4):
                    i = 4 * h_i + q
                    quarter = pg[:, ts(q, K)]
                    nc.tensor.matmul(quarter, ht[:, ts(i, P)], cm2,
                                     start=True, stop=False)
                    nc.tensor.matmul(quarter, ones2, ccr,
                                     start=False, stop=False)
                    nc.tensor.matmul(quarter, hh_sb[0:1, ts(i, P)], oner_r,
                                     start=False, stop=True)
                    # fused gather: distance[b] = psum[b, y_b]
                    tj = work.tile([P, K], f32, tag="tj")
                    nc.vector.scalar_tensor_tensor(
                        out=tj, in0=iota_f, scalar=yf[:, i:i + 1],
                        in1=quarter, op0=ALU.is_equal, op1=ALU.mult,
                        accum_out=NSLN[:, i:i + 1])
                # ln -> exp(0.5) -> exp(-1) chain over the whole half
                nc.scalar.activation(u_t[h_i], pg, AF.Ln)
                nc.scalar.activation(t_t[h_i], u_t[h_i], AF.Exp, scale=0.5)
                nc.scalar.activation(e_t[h_i], t_t[h_i], AF.Exp, scale=-1.0)
                e3 = e_t[h_i].rearrange("p (n k) -> p n k", k=K)
                nc.vector.tensor_reduce(S8[:, ts(h_i, 4)], e3,
                                        axis=AX.X, op=ALU.add)

            # ---- epilogue ----
            nc.scalar.activation(NSLN[:, NT:2 * NT], S8, AF.Ln)
            rm = const.tile([P, KBLK], f32)
            nc.scalar.activation(rm, regmin, AF.Ln)

            final2 = const.tile([P, 2], f32)
            nc.vector.tensor_reduce(final2[:, 0:1], NSLN, axis=AX.X,
                                    op=ALU.add)
            nc.vector.tensor_reduce(final2[:, 1:2], rm, axis=AX.X,
                                    op=ALU.add)
            psf = psm.tile([2, 1], f32, tag="small")
            nc.tensor.matmul(psf, final2, onec_f, start=True, stop=True)
            out_sb = const.tile([2, 1], f32)
            nc.vector.tensor_copy(out_sb, psf)
            nc.sync.dma_start(out=out_d[:, :], in_=out_sb)

    return nc


def _shard_inputs(h, y, centers):
    """Full inputs -> per-core in_maps (host-side shard/replicate/cast)."""
    h = np.asarray(h, dtype=np.float32)
    y = np.asarray(y)
    centers = np.asarray(centers, dtype=np.float32)
    bf = ml_dtypes.bfloat16

    cth = np.ascontiguousarray(centers.T).astype(bf)           # (D, K)
    cc = np.sum(centers.astype(np.float64) ** 2, axis=1).astype(np.float32)
    cc_hi = cc.astype(bf)
    cc_lo = (cc - cc_hi.astype(np.float32)).astype(bf)
    ccr = np.stack([cc_hi, cc_lo]).astype(bf)                  # (2, K)
    cc2 = np.ascontiguousarray(cc.reshape(KBLK, P).T)          # (P, KBLK)

    cth_f32view = np.ascontiguousarray(cth).view(np.float32)   # (P, K//2)
    in_maps = []
    for c in range(N_CORES):
        hs = h[c * BS:(c + 1) * BS]                            # (BS, D)
        ys = y[c * BS:(c + 1) * BS].astype(np.float32)
        ht = np.ascontiguousarray(hs.T).astype(bf)             # (D, BS)
        yf = np.ascontiguousarray(ys.reshape(NT, P).T)         # (P, NT)
        cpk = np.concatenate([cth_f32view, cc2, yf], axis=1)
        in_maps.append({
            "ht": ht, "cpk": np.ascontiguousarray(cpk), "ccr": ccr,
        })
    return in_maps


_NC_CACHE = {}


def kernel(h, y, centers):
    from concourse.bass_utils import run_bass_kernel_spmd

    if "nc" not in _NC_CACHE:
        nc = _build()
        nc.finalize()
        _NC_CACHE["nc"] = nc
    nc = _NC_CACHE["nc"]

    in_maps = _shard_inputs(h, y, centers)
    res = run_bass_kernel_spmd(nc, in_maps, core_ids=list(range(N_CORES)))

    part = np.stack([r["out"].reshape(2) for r in res.results])  # (8, 2)
    loss_mean = float(np.sum(part[:, 0], dtype=np.float64)) / B
    reg = -float(part[0, 1])
    return np.float32(loss_mean + RHO * reg)
